# revision 17
# baseline (speedup 1.0000x reference)
"""Multi-head attention (B=2, N=2048, D=1024, H=16, HD=64) on 8 TRN2 NeuronCores.

Sharding: core c handles batch b = c//4 and heads 4*(c%4) .. 4*(c%4)+3.
Each core computes the QKV projection for its 4 heads, attention, and a
partial output projection (contraction over its 256 hd-columns of w_out).
The host sums the 4 partial outputs per batch (the tensor-parallel
all-reduce) while unsharding.

Schedule design (v4): the kernel is ACT-bound — softmax exp runs only on
the scalar engine: 128 instructions x [128,1024] ~= 140 us. Everything
else is scheduled around keeping that exp stream bubble-free:
  - x-side tensors (xT, wqkv, qkT, v, pt) are bf16 (max rel err ~6e-3 vs
    the 2e-2 gate). w_qkv is host-permuted to [q01|k01|wv|q23|k23] so the
    exp-critical weight columns are one contiguous DMA slice.
  - The input load is COLUMN-staged: stage A (xT[:,0:512] + wqkv[:,0:512],
    2 MiB) is all the first score block needs, so exp #0 fires ~10 us in.
    DMA issue costs ~600 ns/descriptor on a sequencer, so issues rotate
    across the sync/vector/scalar/gpsimd queues.
  - ~26 dummy warm-up matmuls release the HAM clock gate (1.2 -> 2.4 GHz)
    before the first real projection MM.
  - Phase 2 is one flat 128-slot loop (pair-major, qc inner): per slot
    [scores, exp, filler granules, AV(slot-6)]. Deferring AV six slots
    spreads the v-projection crunch of the first 16 slots and removes
    AV bursts at qc boundaries.
  - Remaining projection / output-projection work lives in a deadline-
    ordered filler queue, drained in ~2-matmul granules into slot slack.
  - ACT does exp ONLY; all PSUM drains are DVE; normalization muls run on
    GpSimd (DVE at the tail).
  - softmax needs no max-subtraction (scores are O(few)); the denominator
    rides the AV matmul as a ones-column (M=65); 1/den uses a DRAM
    [1,512]<->[128,4] reshape (single-partition DVE reciprocal is
    ~6ns/elem) and a DRAM broadcast (DMA cannot partition-broadcast from
    SBUF). The last qc skips the reshape (latency over throughput) and
    splits its output projection by pair, accumulating in SBUF, so most
    of it runs before the final normalization.
"""

import os
import sys
import types
import ctypes
import contextlib

import numpy as np
import ml_dtypes
import bass_rust
import concourse.bass as bass
import concourse.tile as tile
from concourse import mybir
from concourse import bass_utils
from concourse.vector_clock import ScopedClock


def _ensure_ntff_hook():
    """Provide antenv.axon_hooks if the container lacks it, so that
    run_bass_kernel_spmd(trace=True) (e.g. via BASS_TRACE=1) works instead
    of raising ModuleNotFoundError."""
    if "antenv.axon_hooks" in sys.modules:
        return
    try:
        import antenv.axon_hooks  # noqa: F401

        return
    except ImportError:
        pass

    def _make_hook():
        so_path = "/opt/axon/libaxon_pjrt.so"
        try:
            lib = ctypes.CDLL(so_path)
        except OSError:
            return None
        if not hasattr(lib, "axon_start_nrt_profile"):
            return None
        lib.axon_start_nrt_profile.argtypes = [
            ctypes.POINTER(ctypes.c_int64),
            ctypes.c_size_t,
        ]
        lib.axon_start_nrt_profile.restype = ctypes.c_int64
        lib.axon_stop_nrt_profile.argtypes = [ctypes.c_char_p]
        lib.axon_stop_nrt_profile.restype = ctypes.c_int64

        @contextlib.contextmanager
        def _hook(output_dir, device_ids):
            import jax

            jax.devices()
            if device_ids:
                ids = (ctypes.c_int64 * len(device_ids))(*device_ids)
                rc = lib.axon_start_nrt_profile(ids, len(device_ids))
            else:
                rc = lib.axon_start_nrt_profile(None, 0)
            if rc != 0:
                raise RuntimeError(f"axon_start_nrt_profile rc={rc}")
            try:
                yield
            finally:
                lib.axon_stop_nrt_profile(str(output_dir).encode())

        return _hook

    hook = _make_hook()
    mod = types.ModuleType("antenv.axon_hooks")
    mod.get_axon_ntff_profile_hook = lambda: hook
    mod.set_axon_ntff_profile_hook = lambda h: None
    sys.modules["antenv.axon_hooks"] = mod


_ensure_ntff_hook()

B, N, D = 2, 2048, 1024
H, HD = 16, 64
HPG = 4  # heads per core
NCORES = 8
ND = D // 128  # 8 contraction chunks for the projections
NT = N // 128  # 16 token/key blocks
NQ = N // 512  # 4 query chunks
DEFER = 8  # AV runs this many slots behind exp

f32 = mybir.dt.float32
f32r = mybir.dt.float32r
bf16 = mybir.dt.bfloat16
EXP = mybir.ActivationFunctionType.Exp

# wqkv column layout (host-permuted): [q01 | k01 | wv(4 heads) | q23 | k23]
RCOL = {0: 0, 2: 128, 1: 512, 3: 640}  # r-block -> wqkv column offset
WVCOL = 256


class _TC(tile.TileContext):
    """TileContext adapted to this walrus build, which encodes at most ONE
    semaphore wait per instruction: excess waits are offloaded onto
    preceding same-engine nops, and the final drain is split the same way."""

    _ws_counter = 0

    def _lower_ordered_insts(self, ordered):
        for bbname, insts in ordered.items():
            new = []
            for inst in insts:
                si = inst.sync_info
                if (
                    si is not None
                    and len(si.on_wait) > 1
                    and inst.engine != mybir.EngineType.Unassigned
                ):
                    waits = list(si.on_wait)
                    ups = list(si.on_update)
                    for w in waits[:-1]:
                        _TC._ws_counter += 1
                        new.append(
                            mybir.InstNoOp(
                                name=f"waitsplit_{_TC._ws_counter}",
                                engine=inst.engine,
                                ins=[],
                                outs=[],
                                sync_info=bass_rust.SyncInfo(
                                    on_wait=[w], on_update=[]
                                ),
                                bass_nofuse=True,
                            )
                        )
                    inst.sync_info = bass_rust.SyncInfo(
                        on_wait=[waits[-1]], on_update=ups
                    )
                new.append(inst)
            ordered[bbname] = new
        super()._lower_ordered_insts(ordered)

    def _drain_and_barrier(self, tick_clock, wait_clock):
        nop0 = self.nc.sync.nop(nofuse=True)
        wait_clock.add_sem_waits(nop0.ins, ScopedClock({None: tick_clock.global_clock}))
        si = nop0.ins.sync_info
        waits = list(si.on_wait) if si is not None else []
        if len(waits) > 1:
            nop0.ins.sync_info = bass_rust.SyncInfo(on_wait=waits[:1], on_update=[])
            for i in range(1, len(waits)):
                n = self.nc.sync.nop(nofuse=True)
                n.ins.sync_info = bass_rust.SyncInfo(
                    on_wait=waits[i : i + 1], on_update=[]
                )
        self.nc.sync.drain()
        self.nc.all_engine_barrier()
        assert self.sems is not None
        popped = self.nc._tile_sem_poison_stack.pop()
        assert popped is self._sem_poison
        self.nc.clear_and_free_semaphores(list(self.sems.allocated().values()))
        self.nc.all_engine_barrier()


class _Filler:
    """Deadline-ordered queue of small PE work granules, drained into the
    slack of each exp slot. Emission order == queue order, so data
    producers are always emitted before their consumers via drain_until."""

    def __init__(self):
        self.q = []  # (key, cost_ns, fn)
        self.done = set()

    def add(self, key, granules):
        for cost, fn in granules:
            self.q.append((key, cost, fn))

    def _pop_front(self):
        key, cost, fn = self.q.pop(0)
        fn()
        if not any(k == key for k, _, _ in self.q):
            self.done.add(key)
        return cost

    def drain_until(self, key):
        if key in self.done or not any(k == key for k, _, _ in self.q):
            return
        while any(k == key for k, _, _ in self.q):
            self._pop_front()

    def drain_budget(self, budget_ns):
        while self.q and budget_ns > 0:
            budget_ns -= self._pop_front()

    def drain_all(self):
        while self.q:
            self._pop_front()


def _body(nc, tc, xT, wqkv, wo, y):
    with contextlib.ExitStack() as ctx:
        persist = ctx.enter_context(tc.tile_pool(name="persist", bufs=1))
        pt_pool = ctx.enter_context(tc.tile_pool(name="ptp", bufs=10))
        ysb_pool = ctx.enter_context(tc.tile_pool(name="ysbp", bufs=4))
        small = ctx.enter_context(tc.tile_pool(name="small", bufs=2))
        dscr = ctx.enter_context(tc.tile_pool(name="dscr", bufs=4, space="DRAM"))
        ps_s = ctx.enter_context(tc.tile_pool(name="ps_s", bufs=2, space="PSUM"))
        ps_av = ctx.enter_context(tc.tile_pool(name="ps_av", bufs=2, space="PSUM"))
        ps_mm = ctx.enter_context(tc.tile_pool(name="ps_mm", bufs=2, space="PSUM"))

        # ---- persistent SBUF residents ----
        xT_sb = [
            persist.tile([128, N], bf16, tag=f"xT{i}", name=f"xT_sb{i}")
            for i in range(ND)
        ]
        wqkv_sb = [
            persist.tile([128, 768], bf16, tag=f"wqkv{i}", name=f"wqkv_sb{i}")
            for i in range(ND)
        ]
        wo_sb = [
            persist.tile([128, D], f32r, tag=f"wo{c2}", name=f"wo_sb{c2}")
            for c2 in range(2)
        ]
        # qkT rows: tile 0 = qT heads 0,1 | tile 1 = qT heads 2,3
        #           tile 2 = kT heads 0,1 | tile 3 = kT heads 2,3
        qkT_sb = [
            persist.tile([128, N], bf16, tag=f"qkT{r}", name=f"qkT_sb{r}")
            for r in range(4)
        ]
        # v blocks with a ones column after each head: [v_h | 1] x 4
        v_sb = [
            persist.tile([128, HPG * (HD + 1)], bf16, tag=f"v{t}", name=f"v_sb{t}")
            for t in range(NT)
        ]
        oT_sb = [
            persist.tile([128, N], f32r, tag=f"oT{c2}", name=f"oT_sb{c2}")
            for c2 in range(2)
        ]
        # pair0 halves of the last qc's output projection, accumulated in SBUF
        y3a = [
            persist.tile([128, 512], f32, tag=f"y3a{j}", name=f"y3a{j}")
            for j in range(8)
        ]

        # ---- PE warm-up: dummy matmuls release the HAM clock gate
        # (4/8 -> 8/8) while stage A of the input load is in flight ----
        warm_sb = persist.tile([128, 512], bf16, tag="warm", name="warm_sb")
        nc.vector.memset(warm_sb, 0.0)
        warm_ps = ps_mm.tile([128, 512], f32, tag="mm", name="warm_ps")
        for _ in range(26):
            nc.tensor.matmul(
                warm_ps, lhsT=warm_sb[:, 0:128], rhs=warm_sb, start=True, stop=True
            )
        for t in range(NT):
            # only the ones-columns need the memset; v_copy writes the rest
            ones = v_sb[t].rearrange("p (h c) -> p h c", c=HD + 1)[:, :, HD : HD + 1]
            nc.vector.memset(ones, 1.0)

        # ---- input DMA, column-staged. Stage A is everything scores kb0-3
        # and v0-3 need; later stages stream in behind the exp stream.
        # Issues rotate across four sequencers (~600 ns per issue each). ----
        issuers = [nc.sync, nc.scalar, nc.gpsimd]
        ii = [0]

        def dma(out, in_, late=False):
            # scalar must be free of issue work before exp#0 (~10 us)
            pool = issuers if not late else [nc.sync, nc.gpsimd]
            eng = pool[ii[0] % len(pool)]
            ii[0] += 1
            eng.dma_start(out=out, in_=in_)

        for i in range(ND):  # stage A
            dma(xT_sb[i][:, 0:512], xT[i * 128 : (i + 1) * 128, 0:512])
            dma(wqkv_sb[i][:, 0:512], wqkv[i * 128 : (i + 1) * 128, 0:512])
        for i in range(ND):  # stage B
            dma(xT_sb[i][:, 512:1024], xT[i * 128 : (i + 1) * 128, 512:1024])

        def late_stages():
            # emitted after the prefix so these issues do not sit ahead of
            # the prefix PSUM-drain copies in the vector queue
            for cb in range(2, 4):
                for i in range(ND):
                    dma(
                        xT_sb[i][:, cb * 512 : (cb + 1) * 512],
                        xT[i * 128 : (i + 1) * 128, cb * 512 : (cb + 1) * 512],
                        late=True,
                    )
            for i in range(ND):
                dma(
                    wqkv_sb[i][:, 512:768],
                    wqkv[i * 128 : (i + 1) * 128, 512:768],
                    late=True,
                )
            for c2 in range(2):
                dma(wo_sb[c2], wo[c2 * 128 : (c2 + 1) * 128, :], late=True)

        # ---- projection-group emitters ----
        def qk_mm(ps, r, qc, i, start, stop):
            c0 = RCOL[r]
            nc.tensor.matmul(
                ps,
                lhsT=wqkv_sb[i][:, c0 : c0 + 128],
                rhs=xT_sb[i][:, qc * 512 : (qc + 1) * 512],
                start=start,
                stop=stop,
            )

        def qk_copy(ps, r, qc):
            nc.vector.tensor_copy(qkT_sb[r][:, qc * 512 : (qc + 1) * 512], ps)

        def v_mm(ps, t, i, start, stop):
            nc.tensor.matmul(
                ps,
                lhsT=xT_sb[i][:, t * 128 : (t + 1) * 128],
                rhs=wqkv_sb[i][:, WVCOL : WVCOL + 256],
                start=start,
                stop=stop,
            )

        def v_copy(ps, t):
            vview = v_sb[t].rearrange("p (h c) -> p h c", c=HD + 1)[:, :, 0:HD]
            nc.vector.tensor_copy(vview, ps.rearrange("p (h c) -> p h c", c=HD))

        # ---- prefix: just the two groups that gate exp #0, chunk-major ----
        pfx0 = ps_mm.tile([128, 512], f32, tag="mm", name="pfx0")
        pfx1 = ps_av.tile([128, 512], f32, tag="av", name="pfx1")
        for i in range(ND):
            qk_mm(pfx0, 0, 0, i, i == 0, i == ND - 1)
            qk_mm(pfx1, 2, 0, i, i == 0, i == ND - 1)
        qk_copy(pfx0, 0, 0)
        qk_copy(pfx1, 2, 0)
        late_stages()

        # ---- filler queue ----
        filler = _Filler()

        def add_qk_group(r, qc):
            ps_box = []

            def mk(i0, i1, start, stop, last):
                def fn():
                    if start:
                        ps_box.append(
                            ps_mm.tile([128, 512], f32, tag="mm", name=f"qkg_{r}_{qc}")
                        )
                    for i in range(i0, i1):
                        qk_mm(ps_box[0], r, qc, i, start and i == i0, stop and i == i1 - 1)
                    if last:
                        qk_copy(ps_box[0], r, qc)

                return fn

            filler.add(
                ("qk", r, qc),
                [
                    (426, mk(0, 2, True, False, False)),
                    (426, mk(2, 4, False, False, False)),
                    (426, mk(4, 6, False, False, False)),
                    (426, mk(6, 8, False, True, True)),
                ],
            )

        def add_v_group(t):
            ps_box = []

            def mk(i0, i1, start, stop, last):
                def fn():
                    if start:
                        ps_box.append(
                            ps_mm.tile([128, HPG * HD], f32, tag="mm", name=f"vg_{t}")
                        )
                    for i in range(i0, i1):
                        v_mm(ps_box[0], t, i, start and i == i0, stop and i == i1 - 1)
                    if last:
                        v_copy(ps_box[0], t)

                return fn

            filler.add(
                ("v", t),
                [
                    (426, mk(0, 4, True, False, False)),
                    (426, mk(4, 8, False, True, True)),
                ],
            )

        def outproj_piece(t, dc):
            ps = ps_mm.tile([128, 512], f32, tag="mm", name=f"ps_y_{t}_{dc}")
            for c2 in range(2):
                nc.tensor.matmul(
                    ps,
                    lhsT=oT_sb[c2][:, t * 128 : (t + 1) * 128],
                    rhs=wo_sb[c2][:, dc * 512 : (dc + 1) * 512],
                    start=(c2 == 0),
                    stop=(c2 == 1),
                )
            ysb = ysb_pool.tile([128, 512], f32, tag="y", name=f"ysb_{t}_{dc}")
            nc.vector.tensor_copy(ysb, ps)
            nc.sync.dma_start(
                out=y[t * 128 : (t + 1) * 128, dc * 512 : (dc + 1) * 512],
                in_=ysb,
            )

        def add_outproj(qc):
            filler.add(
                ("op", qc),
                [
                    (500, lambda t=qc * 4 + tb, dc=dc: outproj_piece(t, dc))
                    for tb in range(4)
                    for dc in range(2)
                ],
            )

        def op3a_piece(j):
            # pair0 half of the last qc's output projection -> SBUF
            t, dc = NT - 4 + j // 2, j % 2
            ps = ps_mm.tile([128, 512], f32, tag="mm", name=f"ps_y3a_{j}")
            nc.tensor.matmul(
                ps,
                lhsT=oT_sb[0][:, t * 128 : (t + 1) * 128],
                rhs=wo_sb[0][:, dc * 512 : (dc + 1) * 512],
                start=True,
                stop=True,
            )
            nc.vector.tensor_copy(y3a[j], ps)

        def add_op3a():
            filler.add(
                ("op3a",), [(500, lambda j=j: op3a_piece(j)) for j in range(8)]
            )

        # deadline order (slot): r2qc1@4, v0@6, v1@7, r2qc2@8, v2.., r2qc3@12
        add_qk_group(2, 1)
        add_v_group(0)
        add_v_group(1)
        add_qk_group(2, 2)
        for t in range(2, 5):
            add_v_group(t)
        add_qk_group(2, 3)
        for t in range(5, 10):
            add_v_group(t)
        add_qk_group(0, 1)
        for t in range(10, NT):
            add_v_group(t)
        add_qk_group(0, 2)
        add_qk_group(0, 3)
        add_qk_group(1, 0)
        add_qk_group(3, 0)
        add_qk_group(3, 1)
        add_qk_group(3, 2)
        add_qk_group(3, 3)
        add_qk_group(1, 1)
        add_qk_group(1, 2)
        add_qk_group(1, 3)

        # ---- phase 2: flat 128-slot exp-stream loop with AV deferred ----
        def av_pair(pair, poA, poB, kb, pt):
            hA, hB = 2 * pair, 2 * pair + 1
            nc.tensor.matmul(
                poA,
                lhsT=v_sb[kb][:, hA * (HD + 1) : (hA + 1) * (HD + 1)],
                rhs=pt[:, 0:512],
                start=(kb == 0),
                stop=(kb == NT - 1),
            )
            nc.tensor.matmul(
                poB,
                lhsT=v_sb[kb][:, hB * (HD + 1) : (hB + 1) * (HD + 1)],
                rhs=pt[:, 512:1024],
                start=(kb == 0),
                stop=(kb == NT - 1),
            )

        def emit_norm(pair, qc, poA, poB, tail=False):
            """Drain [o|den], compute 1/den, scale oT. Muls on GpSimd
            (DVE at the tail); tail skips the [128,4] reshape hops."""
            oaccs = []
            for h, po in ((2 * pair, poA), (2 * pair + 1, poB)):
                oacc = small.tile(
                    [65, 512], f32, tag="oacc", name=f"oacc_{pair}_{qc}_{h}", bufs=4
                )
                nc.vector.tensor_copy(oacc, po)
                oaccs.append((h, oacc))
            for h, oacc in oaccs:
                qb = (h % 2) * 64
                scr2 = dscr.tile([1, 512], f32, tag="scr2", name=f"scr2_{pair}_{qc}_{h}")
                if tail:
                    rd = small.tile([1, 512], f32, tag="rd", name=f"rd_{pair}_{qc}_{h}")
                    nc.vector.reciprocal(rd, oacc[64:65, :])
                    nc.sync.dma_start(out=scr2, in_=rd)
                else:
                    scr = dscr.tile([1, 512], f32, tag="scr", name=f"scr_{pair}_{qc}_{h}")
                    nc.sync.dma_start(out=scr, in_=oacc[64:65, :])
                    rin = small.tile([128, 4], f32, tag="rin", name=f"rin_{pair}_{qc}_{h}")
                    nc.sync.dma_start(
                        out=rin, in_=scr.rearrange("a (p c) -> (a p) c", c=4)
                    )
                    rout = small.tile(
                        [128, 4], f32, tag="rout", name=f"rout_{pair}_{qc}_{h}"
                    )
                    nc.vector.reciprocal(rout, rin)
                    nc.sync.dma_start(
                        out=scr2.rearrange("a (p c) -> (a p) c", c=4), in_=rout
                    )
                rep = small.tile([64, 512], f32, tag="rep", name=f"rep_{pair}_{qc}_{h}")
                nc.sync.dma_start(out=rep, in_=scr2.to_broadcast((64, 512)))
                mul_eng = nc.vector if tail else nc.gpsimd
                mul_eng.tensor_mul(
                    oT_sb[pair][qb : qb + 64, qc * 512 : (qc + 1) * 512],
                    oacc[0:64, :],
                    rep,
                )

        LOOPS = [(pair, qc) for pair in range(2) for qc in range(NQ)]
        pend = []  # FIFO of (pair, poA, poB, kb, pt, qc)
        po_of = {}

        def flush_one():
            pair, poA, poB, kb, pt, qc = pend.pop(0)
            filler.drain_until(("v", kb))
            av_pair(pair, poA, poB, kb, pt)
            if kb == NT - 1:
                emit_norm(pair, qc, poA, poB, tail=(pair, qc) == LOOPS[-1])
                if pair == 0 and qc == NQ - 1:
                    # pair0 halves of qc3's outproj only need pair0's oT
                    add_op3a()
                if pair == 1 and qc < NQ - 1:
                    add_outproj(qc)

        for li, (pair, qc) in enumerate(LOOPS):
            poA = ps_av.tile([65, 512], f32, tag="av", name=f"poA_{pair}_{qc}")
            poB = ps_av.tile([65, 512], f32, tag="av", name=f"poB_{pair}_{qc}")
            po_of[(pair, qc)] = (poA, poB)
            for kb in range(NT):
                filler.drain_until(("qk", 2 + pair, kb // 4))
                if kb == 0:
                    filler.drain_until(("qk", pair, qc))
                ps = ps_s.tile([128, 1024], f32, tag="s", name=f"ps_s_{pair}_{qc}_{kb}")
                nc.tensor.matmul(
                    ps[:, 0:512],
                    lhsT=qkT_sb[2 + pair][0:64, kb * 128 : (kb + 1) * 128],
                    rhs=qkT_sb[pair][0:64, qc * 512 : (qc + 1) * 512],
                    start=True,
                    stop=True,
                )
                nc.tensor.matmul(
                    ps[:, 512:1024],
                    lhsT=qkT_sb[2 + pair][64:128, kb * 128 : (kb + 1) * 128],
                    rhs=qkT_sb[pair][64:128, qc * 512 : (qc + 1) * 512],
                    start=True,
                    stop=True,
                )
                pt = pt_pool.tile([128, 1024], bf16, tag="pt", name=f"pt_{pair}_{qc}_{kb}")
                nc.scalar.activation(pt, ps, EXP, scale=HD**-0.5)
                filler.drain_budget(500)
                pend.append((pair, poA, poB, kb, pt, qc))
                if len(pend) > DEFER:
                    flush_one()
        while pend:
            flush_one()
        filler.drain_all()
        assert not filler.q

        # ---- tail: pair1 halves of the last qc's output projection,
        # accumulated against the SBUF-held pair0 halves ----
        for j in range(8):
            t, dc = NT - 4 + j // 2, j % 2
            ps = ps_mm.tile([128, 512], f32, tag="mm", name=f"ps_y3b_{j}")
            nc.tensor.matmul(
                ps,
                lhsT=oT_sb[1][:, t * 128 : (t + 1) * 128],
                rhs=wo_sb[1][:, dc * 512 : (dc + 1) * 512],
                start=True,
                stop=True,
            )
            ysb = ysb_pool.tile([128, 512], f32, tag="y", name=f"ysb3_{j}")
            nc.vector.tensor_add(ysb, ps, y3a[j])
            nc.sync.dma_start(
                out=y[t * 128 : (t + 1) * 128, dc * 512 : (dc + 1) * 512],
                in_=ysb,
            )


def build():
    nc = bass.Bass("TRN2", target_bir_lowering=False)
    xT = nc.dram_tensor("xT", [D, N], bf16, kind="ExternalInput").ap()
    wqkv = nc.dram_tensor("wqkv", [D, 768], bf16, kind="ExternalInput").ap()
    wo = nc.dram_tensor("wo", [HPG * HD, D], f32r, kind="ExternalInput").ap()
    y = nc.dram_tensor("y", [N, D], f32, kind="ExternalOutput").ap()
    with _TC(nc) as tc:
        _body(nc, tc, xT, wqkv, wo, y)
    return nc


def shard_inputs(x, w_qkv, w_out):
    """Build the 8 per-core input maps from the full tensors."""
    x = np.asarray(x, dtype=np.float32)
    w_qkv = np.asarray(w_qkv, dtype=np.float32)
    w_out = np.asarray(w_out, dtype=np.float32)
    bf = ml_dtypes.bfloat16
    in_maps = []
    for c in range(NCORES):
        b, grp = c // 4, c % 4
        heads = [HPG * grp + i for i in range(HPG)]
        xTa = np.ascontiguousarray(x[b].T.astype(bf))
        qcols = [w_qkv[:, h * HD : (h + 1) * HD] for h in heads]
        kcols = [w_qkv[:, H * HD + h * HD : H * HD + (h + 1) * HD] for h in heads]
        vcols = [w_qkv[:, 2 * H * HD + h * HD : 2 * H * HD + (h + 1) * HD] for h in heads]
        # column layout [q01 | k01 | wv(4 heads) | q23 | k23]
        wqkv_a = np.ascontiguousarray(
            np.concatenate(
                qcols[0:2] + kcols[0:2] + vcols + qcols[2:4] + kcols[2:4], axis=1
            ).astype(bf)
        )
        wo_a = np.ascontiguousarray(
            np.concatenate([w_out[h * HD : (h + 1) * HD, :] for h in heads], axis=0)
        )
        in_maps.append({"xT": xTa, "wqkv": wqkv_a, "wo": wo_a})
    return in_maps


LAST_RESULTS = None  # BassKernelResults from the most recent kernel() call
_NC_CACHE = None


def kernel(x, w_qkv, w_out):
    global LAST_RESULTS, _NC_CACHE
    if _NC_CACHE is None:
        _NC_CACHE = build()
    nc = _NC_CACHE
    in_maps = shard_inputs(x, w_qkv, w_out)
    trace = bool(os.environ.get("KERNEL_TRACE"))
    res = bass_utils.run_bass_kernel_spmd(
        nc, in_maps, core_ids=list(range(NCORES)), trace=trace
    )
    LAST_RESULTS = res
    y = np.zeros((B, N, D), dtype=np.float32)
    for c in range(NCORES):
        y[c // 4] += res.results[c]["y"]
    return y
